# revision 4
# baseline (speedup 1.0000x reference)
"""DynamicConv1D Trainium2 kernel.

Reference computation (per batch b):
  dw = conv1d(x, W, pad=3) + b            # [O*I*K, T] dynamic weights
  dw = softmax(dw.reshape(O,I,K,T)/sqrt(K), axis=K)
  y[o,t] = sum_{i,k} x[i, t+k-3] * dw[o,i,k,t]

Sharding: 8 cores = 4 batches x 2 halves of O (16 out-channels each).
Each core gets x[b] plus its half of the (rearranged) conv weights and
computes y[b, half*16:(half+1)*16, :]. No collectives; the host scatters
inputs and concatenates outputs.

Per-core layout (t-tile = 128 positions on partitions):
  conv as matmul: dw[t, (k,o,i)] = sum_{(j,c)} X1[(j,c), t] * W'[(j,c), (k,o,i)]
    X1[(j,c), u] = x[c, u+j-3]  (im2col layout built host-side, bf16)
    ones row appended to X1 so the bias rides as an extra W' row;
    1/sqrt(K) is folded into W' and b on the host.
  PE schedule: 8 chunks of 448 cols per tile, grouped 4+4 into two 4-bank
    PSUM groups that ping-pong; per group the stationary x1 block loads
    twice (a/b contraction halves) and 8 matmuls run back-to-back so the
    tensor engine stays at its high p-state.
  ScalarE drains each 4-bank group with one wide exp -> bf16 e in SBUF.
  DVE tail, all bf16 contiguous (2-elem-packed SBUF operands run 4x):
    ex = e * x2rep          (x2rep = x_unf replicated over o, shipped bf16)
    den/num = sum_k {e,ex}  (batched adds on the [TT,2,*] eex view)
    y[t,o] = sum_i num/den  (tensor_tensor divide + grouped reduce)
"""

import numpy as np

B = 4
C = 32
K = 7
T = 4096
O_FULL = 32
OH = 16  # out-channels per core
PAD = 3
TT = 128  # t positions per tile (partition dim)
FREE = K * OH * C  # 3584, free index = k*512 + o*32 + i
SLAB = OH * C  # 512, one k-slab
CD1 = 128  # (j, c) rows for j=0..3
CD2 = 97  # (j, c) rows for j=4..6 plus ones row
MCH = 448  # matmul chunk (cols); 8 per tile, 4 per psum group
GRP = 4 * MCH  # 1792 free elems drained per activation

_prog_cache = {}


def _build(t_len):
    """Build and compile the per-core Bass program for sequence length t_len."""
    import concourse.tile as tile
    from concourse import bacc, mybir

    nt = t_len // TT
    nc = bacc.Bacc("TRN2", target_bir_lowering=False, debug=False, num_devices=1)
    f32 = mybir.dt.float32
    bf16 = mybir.dt.bfloat16

    x1a_d = nc.dram_tensor("x1a", [CD1, t_len], bf16, kind="ExternalInput").ap()
    x1b_d = nc.dram_tensor("x1b", [CD2, t_len], bf16, kind="ExternalInput").ap()
    w1_d = nc.dram_tensor("wp1", [CD1, FREE], bf16, kind="ExternalInput").ap()
    w2_d = nc.dram_tensor("wp2", [CD2, FREE], bf16, kind="ExternalInput").ap()
    x2r_d = nc.dram_tensor("x2r", [TT, nt * FREE], bf16, kind="ExternalInput").ap()
    y_d = nc.dram_tensor("yout", [TT, nt * OH], f32, kind="ExternalOutput").ap()

    with tile.TileContext(nc) as tc:
        with (
            tc.tile_pool(name="const", bufs=1) as cpool,
            tc.tile_pool(name="x2p", bufs=3) as x2pool,
            tc.tile_pool(name="ep", bufs=3) as epool,
            tc.tile_pool(name="tree", bufs=2) as tpool,
            tc.tile_pool(name="small", bufs=2) as spool,
            tc.tile_pool(name="pair", bufs=2) as qpool,
            tc.tile_pool(name="psum", bufs=2, space="PSUM") as ppool,
        ):
            x1a_bf = cpool.tile([CD1, t_len], bf16, tag="x1abf")
            x1b_bf = cpool.tile([CD2, t_len], bf16, tag="x1bbf")
            w1_bf = cpool.tile([CD1, FREE], bf16, tag="w1bf")
            w2_bf = cpool.tile([CD2, FREE], bf16, tag="w2bf")
            y_sb = cpool.tile([TT, nt * OH], f32, tag="ysb")

            # Constant loads. The PE needs x1a + the first w columns first;
            # x1b/w2 ride the gpsimd queue (which later streams x2rep).
            hf = FREE // 2
            nc.sync.dma_start(x1a_bf[:], x1a_d[:])
            nc.gpsimd.dma_start(x1b_bf[:], x1b_d[:])
            nc.sync.dma_start(w1_bf[:, 0:hf], w1_d[:, 0:hf])
            nc.gpsimd.dma_start(w2_bf[:, 0:hf], w2_d[:, 0:hf])
            nc.sync.dma_start(w1_bf[:, hf:], w1_d[:, hf:])
            nc.gpsimd.dma_start(w2_bf[:, hf:], w2_d[:, hf:])

            for tt in range(nt):
                t0 = tt * TT
                # x_unf replicated over o, built host-side: [TT, (k,o,i)]
                x2r = x2pool.tile([TT, FREE], bf16, tag="x2r")
                nc.gpsimd.dma_start(
                    x2r[:], x2r_d[:, tt * FREE : (tt + 1) * FREE]
                )

                # eex holds e (exp of dw) and EX (e * x_unf) side by side so
                # the den/num k-sum trees batch into single wide DVE ops.
                eex = epool.tile([TT, 2, FREE], bf16, tag="eex")
                e = eex[:, 0]

                for g in range(2):
                    pg = ppool.tile([TT, 4 * SLAB], f32, tag="pg", name="pg")
                    pgv = pg[:].rearrange("p (a b) -> p a b", a=4)
                    for c in range(4):
                        cs = slice((4 * g + c) * MCH, (4 * g + c + 1) * MCH)
                        nc.tensor.matmul(
                            pgv[:, c, 0:MCH], x1a_bf[:, t0 : t0 + TT],
                            w1_bf[:, cs], start=True, stop=False,
                        )
                    for c in range(4):
                        cs = slice((4 * g + c) * MCH, (4 * g + c + 1) * MCH)
                        nc.tensor.matmul(
                            pgv[:, c, 0:MCH], x1b_bf[:, t0 : t0 + TT],
                            w2_bf[:, cs], start=False, stop=True,
                        )
                    dst = e[:, g * GRP : (g + 1) * GRP].rearrange(
                        "p (a b) -> p a b", a=4
                    )
                    nc.scalar.activation(
                        dst, pgv[:, :, 0:MCH], mybir.ActivationFunctionType.Exp
                    )

                # EX = e * x_unf (both packed bf16 -> 4x DVE mode)
                nc.vector.tensor_mul(eex[:, 1], e, x2r[:])

                # k-sum trees for den (over e) and num (over EX), batched as
                # one wide op per level via the [TT, 2, ...] eex view.
                # Slabs k0..k6; L1: (k0..2)+(k3..5), L2a/L2b/L3 finish, with
                # k6 folded in at L2b. All operands packed bf16.
                ev = eex[:]
                t1 = tpool.tile([TT, 2, 3 * SLAB], bf16, tag="t1")
                nc.vector.tensor_add(
                    t1[:], ev[:, :, 0 : 3 * SLAB], ev[:, :, 3 * SLAB : 6 * SLAB]
                )
                t2 = spool.tile([TT, 2, 2, SLAB], bf16, tag="t2")
                nc.vector.tensor_add(
                    t2[:, :, 0], t1[:, :, 0:SLAB], t1[:, :, SLAB : 2 * SLAB]
                )
                nc.vector.tensor_add(
                    t2[:, :, 1], t1[:, :, 2 * SLAB :], ev[:, :, 6 * SLAB :]
                )
                # num stays bf16; den goes to f32 for the fast reciprocal.
                if tt % 2 == 0:
                    dnum = qpool.tile([TT, 2, SLAB], bf16, tag="dnum")
                    denf = qpool.tile([TT, 2, SLAB], f32, tag="denf")
                nc.vector.tensor_add(dnum[:, tt % 2], t2[:, 1, 0], t2[:, 1, 1])
                nc.vector.tensor_add(denf[:, tt % 2], t2[:, 0, 0], t2[:, 0, 1])

                if tt % 2 == 1:
                    # softmax tail for the tile pair: 1/den, then
                    # y[t,o] = sum_i num * r
                    r = qpool.tile([TT, 2, SLAB], f32, tag="r")
                    nc.vector.reciprocal_approx_fast(out=r[:], in_=denf[:])
                    y1 = qpool.tile([TT, 2, SLAB], bf16, tag="y1")
                    nc.vector.tensor_mul(y1[:], dnum[:], r[:])
                    nc.vector.tensor_reduce(
                        y_sb[:, (tt - 1) * OH : (tt + 1) * OH],
                        y1[:].rearrange("p u (o i) -> p u o i", o=OH),
                        axis=mybir.AxisListType.X,
                        op=mybir.AluOpType.add,
                    )

                if (tt + 1) % 8 == 0 or tt == nt - 1:
                    g0 = (tt // 8) * 8 * OH
                    nc.sync.dma_start(
                        y_d[:, g0 : (tt + 1) * OH], y_sb[:, g0 : (tt + 1) * OH]
                    )

    nc.compile()
    return nc


def _prep_inputs(x, W, b):
    """Host-side scatter: per-core input dicts (pure layout/slicing)."""
    import ml_dtypes

    bf = ml_dtypes.bfloat16
    scale = np.float32(1.0 / np.sqrt(K))
    halves = []
    for h in range(2):
        Wh = W[h * OH * C * K : (h + 1) * OH * C * K]  # [OH*C*K, C, K]
        # rows (j,c) -> j*32+c ; cols (k,o,i) -> k*512 + o*32 + i
        Wp = (
            Wh.reshape(OH, C, K, C, K).transpose(4, 3, 2, 0, 1).reshape(K * C, FREE)
            * scale
        )
        bh = (
            b[h * OH * C * K : (h + 1) * OH * C * K]
            .reshape(OH, C, K)
            .transpose(2, 0, 1)
            .reshape(FREE)
            * scale
        )
        w1 = np.ascontiguousarray(Wp[:CD1])
        w2 = np.ascontiguousarray(
            np.concatenate([Wp[CD1:], bh[None, :]], axis=0)
        )
        halves.append((w1.astype(bf), w2.astype(bf)))

    t_len = x.shape[-1]
    nt = t_len // TT
    x1s = []
    x2rs = []
    for bi in range(B):
        xp = np.zeros((C, t_len + 2 * PAD), dtype=np.float32)
        xp[:, PAD : PAD + t_len] = x[bi]
        x1a = np.empty((CD1, t_len), dtype=np.float32)
        x1b = np.empty((CD2, t_len), dtype=np.float32)
        for j in range(K):
            tgt, r0 = (x1a, j * C) if j < 4 else (x1b, (j - 4) * C)
            tgt[r0 : r0 + C] = xp[:, j : j + t_len]
        x1b[CD2 - 1] = 1.0
        x1s.append((x1a.astype(bf), x1b.astype(bf)))

        # x2rep[t, k, o, i] = xp[i, t+k], replicated over o, laid out so each
        # t-tile is one contiguous [TT, FREE] DMA.
        xpT = np.ascontiguousarray(xp.T).astype(bf)  # [t_len+6, C]
        win = np.lib.stride_tricks.sliding_window_view(xpT, K, axis=0)
        # win[t, i, k] = xpT[t+k, i]
        x2 = win.transpose(0, 2, 1)  # [t_len, K, C]
        x2rep = np.broadcast_to(
            x2[:, :, None, :], (t_len, K, OH, C)
        ).reshape(nt, TT, FREE)
        x2rs.append(
            np.ascontiguousarray(
                x2rep.transpose(1, 0, 2).reshape(TT, nt * FREE)
            )
        )

    in_maps = []
    for core in range(8):
        bi, h = divmod(core, 2)
        w1, w2 = halves[h]
        x1a, x1b = x1s[bi]
        in_maps.append(
            {"x1a": x1a, "x1b": x1b, "wp1": w1, "wp2": w2, "x2r": x2rs[bi]}
        )
    return in_maps


def _assemble(results, t_len):
    """Gather per-core [TT, nt*OH] outputs into [B, O_FULL, t_len]."""
    nt = t_len // TT
    y = np.empty((B, O_FULL, t_len), dtype=np.float32)
    for core, res in enumerate(results):
        bi, h = divmod(core, 2)
        arr = res["yout"].reshape(TT, nt, OH)  # [tp, tt, o]
        y[bi, h * OH : (h + 1) * OH, :] = arr.transpose(2, 1, 0).reshape(OH, t_len)
    return y


def _run(x, W, b, trace=False, trace_cores=None):
    from concourse.bass_utils import run_bass_kernel_spmd
    from concourse.bass_interp import get_hw_module

    t_len = x.shape[-1]
    key = ("prog", t_len)
    if key not in _prog_cache:
        nc = _build(t_len)
        nc.m = get_hw_module(nc.m)
        _prog_cache[key] = nc
    nc = _prog_cache[key]

    in_maps = _prep_inputs(x, W, b)
    res = run_bass_kernel_spmd(
        nc,
        in_maps,
        core_ids=list(range(8)),
        trace=trace,
        trace_cores=trace_cores,
    )
    return _assemble(res.results, t_len), res


def kernel(x, W, b):
    y, _ = _run(np.asarray(x), np.asarray(W), np.asarray(b))
    return y
